# revision 15
# baseline (speedup 1.0000x reference)
"""Trainium2 Bass kernel for nn_DGCLoss (DCG/NDCG ranking loss).

Math restructure (vs. the reference's [N, M, M-1] cube):

For row n, with s = off-diag cosine-sim row mapped to [0,1]:
    indicator[n,i] = 1 + sum_{j != i} sigmoid((s_j - s_i)/K)
Working instead with the FULL 384-vector of cosines c (diag masked to -1e6):
    T(i) = sum_{j=0..N-1} sigmoid(500*(c_j - c_i))      # diag j=n contributes 0
         = indicator[n,i] - 0.5                         # (j=i term = 0.5)
Using sigmoid(z) = 0.5 + 0.5*tanh(z/2):
    A(i) = sum_j tanh(250*(c_j - c_i));  u = indicator+1 = 0.5*A + N/2 + 1.5
    dcg[n] = sum_i rel0[n,i] * ln2 / ln(u(i))           # rel0 has zero diag

Per core (8 cores, 48 rows each):
  - PE: gram slice (fp32r), s~ transposes, one broadcast matmul per row
        (one-hot weights x s~ tile -> PSUM[p,j] = 250*c_j), final reduction.
  - ACT: 3 tanh instructions per row ([128,384], per-partition bias
        -250*c_i, accum_out = row sums) -> A tiles; tail ln.
  - DVE: diag mask add, bias scaling, reciprocal, relevance multiply.
idcg depends only on gt[n] (<= 6 distinct values): computed on host.
"""

import math

import numpy as np

N = 384
D = 256
NCORES = 8
RPC = N // NCORES  # 48 rows per core
EPS = 1e-8
NEG_BIG = -1.0e6
LN2 = math.log(2.0)

_CACHE = {}


# ---------------------------------------------------------------- device code


def _build_nc():
    """Build + compile the (SPMD, per-core) Bass program."""
    from contextlib import ExitStack

    import concourse.bacc as bacc
    import concourse.mybir as mybir
    import concourse.tile as tile

    f32 = mybir.dt.float32
    AF = mybir.ActivationFunctionType

    nc = bacc.Bacc(
        "TRN2",
        target_bir_lowering=False,
        debug=False,
        enable_asserts=True,
        num_devices=NCORES,
    )

    xnt_d = nc.dram_tensor("xnt", [D, N], f32, kind="ExternalInput")
    xst_d = nc.dram_tensor("xst", [D, RPC], f32, kind="ExternalInput")
    i48_d = nc.dram_tensor("i48", [RPC, RPC], f32, kind="ExternalInput")
    ones_d = nc.dram_tensor("ones128", [128, 1], f32, kind="ExternalInput")
    dmask_d = nc.dram_tensor("dmask", [RPC, N], f32, kind="ExternalInput")
    relt_d = nc.dram_tensor("relt", [N, RPC], f32, kind="ExternalInput")
    dcg_d = nc.dram_tensor("dcg", [1, RPC], f32, kind="ExternalOutput")

    with tile.TileContext(nc) as tc, ExitStack() as ctx:
        const = ctx.enter_context(tc.tile_pool(name="const", bufs=1))
        junkp = ctx.enter_context(tc.tile_pool(name="junk", bufs=2))

        xnt_sb = []
        for k in range(2):
            t = const.tile([128, N], f32, name=f"xnt_sb{k}", tag=f"xnt{k}")
            nc.sync.dma_start(t[:], xnt_d.ap()[128 * k : 128 * (k + 1), :])
            xnt_sb.append(t)
        xst_sb = []
        for k in range(2):
            t = const.tile([128, RPC], f32, name=f"xst_sb{k}", tag=f"xst{k}")
            nc.sync.dma_start(t[:], xst_d.ap()[128 * k : 128 * (k + 1), :])
            xst_sb.append(t)
        i48_sb = const.tile([RPC, RPC], f32, name="i48_sb", tag="i48")
        nc.sync.dma_start(i48_sb[:], i48_d.ap()[:])
        # one-hot broadcast weights built on device: W[r, 128r:128(r+1)] = 250
        # via W_block_r = 250 * i48[:, r]  (per-partition scalar multiply)
        w250_sb = const.tile([RPC, RPC * 128], f32, name="w250_sb", tag="w250")
        c250_sb = const.tile([RPC, 128], f32, name="c250_sb", tag="c250")
        nc.vector.memset(c250_sb[:], 250.0)
        for r in range(RPC):
            nc.vector.tensor_scalar_mul(
                w250_sb[:, 128 * r : 128 * (r + 1)],
                c250_sb[:],
                i48_sb[:, r : r + 1],
            )
        ones_sb = const.tile([128, 1], f32, name="ones_sb", tag="ones")
        nc.sync.dma_start(ones_sb[:], ones_d.ap()[:])
        dmask_sb = const.tile([RPC, N], f32, name="dmask_sb", tag="dmask")
        nc.sync.dma_start(dmask_sb[:], dmask_d.ap()[:])
        relt_sb = []
        for c in range(3):
            t = const.tile([128, RPC], f32, name=f"relt_sb{c}", tag=f"relt{c}")
            nc.sync.dma_start(t[:], relt_d.ap()[128 * c : 128 * (c + 1), :])
            relt_sb.append(t)

        s_sb = const.tile([RPC, N], f32, name="s_sb", tag="s")
        bt_sb = [
            const.tile([128, RPC], f32, name=f"bt_sb{c}", tag=f"bt{c}")
            for c in range(3)
        ]
        a_sb = [
            const.tile([128, RPC], f32, name=f"a_sb{c}", tag=f"a{c}")
            for c in range(3)
        ]

        # ---- phase 1: gram slice, diag mask, transposed bias tiles
        with tc.tile_pool(name="pg", bufs=1, space="PSUM") as pgp, tc.tile_pool(
            name="pt", bufs=1, space="PSUM"
        ) as ptp:
            pg = pgp.tile([RPC, N], f32, name="pg", tag="pg")
            nc.tensor.matmul(
                pg[:],
                xst_sb[0][:],
                xnt_sb[0][:],
                start=True,
                stop=False,
            )
            nc.tensor.matmul(
                pg[:],
                xst_sb[1][:],
                xnt_sb[1][:],
                start=False,
                stop=True,
            )
            # s~ = cosine row block + diagonal mask (-1e6 at global diag)
            nc.vector.tensor_add(s_sb[:], pg[:], dmask_sb[:])
            for c in range(3):
                pt = ptp.tile([128, RPC], f32, name=f"pt{c}", tag=f"pt{c}")
                nc.tensor.transpose(
                    pt[:], s_sb[0:RPC, 128 * c : 128 * (c + 1)], i48_sb[:]
                )
                # bias = -250 * s~^T  (per-partition bias for the tanh)
                nc.vector.tensor_scalar_mul(bt_sb[c][:], pt[:], -250.0)

        # ---- phase 2: main loop. Per row r: one broadcast matmul + 3 tanh
        last_tanh = None
        with tc.tile_pool(name="pb", bufs=6, space="PSUM") as pbp:
            for r in range(RPC):
                pb = pbp.tile([128, N], f32, name=f"pb{r}", tag="pb")
                # pb[p, j] = 250 * s~[r, j]  (one-hot row r of w250)
                nc.tensor.matmul(
                    pb[:],
                    w250_sb[:, 128 * r : 128 * (r + 1)],
                    s_sb[:],
                    start=True,
                    stop=True,
                )
                for c in range(3):
                    jt = junkp.tile([128, N], f32, name=f"jt{r}_{c}", tag="junk")
                    last_tanh = nc.scalar.activation(
                        jt[:],
                        pb[:],
                        AF.Tanh,
                        bias=bt_sb[c][:, r : r + 1],
                        scale=1.0,
                        accum_out=a_sb[c][:, r : r + 1],
                    )

        # ---- phase 3: tail.  u = 0.5*A + (N/2 + 1.5); dcg = sum_i relt/ln(u)
        with tc.tile_pool(name="pd", bufs=1, space="PSUM") as pdp:
            pd = pdp.tile([1, RPC], f32, name="pd", tag="pd")
            lnu = [
                const.tile([128, RPC], f32, name=f"lnu{c}", tag=f"lnu{c}")
                for c in range(3)
            ]
            dterm = [
                const.tile([128, RPC], f32, name=f"dterm{c}", tag=f"dterm{c}")
                for c in range(3)
            ]
            ubias = const.tile([128, 1], f32, name="ubias", tag="ubias")
            nc.vector.memset(ubias[:], float(N / 2 + 1.5))
            for c in range(3):
                ln_inst = nc.scalar.activation(
                    lnu[c][:],
                    a_sb[c][:],
                    AF.Ln,
                    bias=ubias[:],
                    scale=0.5,
                )
                # keep the Ln (different ACT table set) strictly after every
                # tanh so only one table swap happens
                tile.add_dep_helper(
                    ln_inst.ins,
                    last_tanh.ins,
                    reason="batch ACT table sets: all tanh before ln",
                )
                nc.vector.reciprocal(lnu[c][:], lnu[c][:])
                nc.vector.tensor_mul(dterm[c][:], lnu[c][:], relt_sb[c][:])
                nc.tensor.matmul(
                    pd[:],
                    ones_sb[:],
                    dterm[c][:],
                    start=(c == 0),
                    stop=(c == 2),
                )
            out_sb = const.tile([1, RPC], f32, name="out_sb", tag="out")
            nc.vector.tensor_copy(out_sb[:], pd[:])
            nc.sync.dma_start(dcg_d.ap()[:], out_sb[:])

    nc.compile()
    return nc


def _get_nc():
    if "nc" not in _CACHE:
        _CACHE["nc"] = _build_nc()
    return _CACHE["nc"]


# ------------------------------------------------------------------ execution


def _get_runner():
    """Cached jitted 8-core SPMD executor (modeled on bass2jax's
    run_bass_via_pjrt multi-core path, but reusable across calls)."""
    if "runner" in _CACHE:
        return _CACHE["runner"]

    import jax
    from jax.sharding import Mesh, PartitionSpec
    from jax.experimental.shard_map import shard_map

    import concourse.mybir as mybir
    from concourse.bass2jax import (
        _bass_exec_p,
        install_neuronx_cc_hook,
        partition_id_tensor,
    )

    nc = _get_nc()
    install_neuronx_cc_hook()

    partition_name = (
        nc.partition_id_tensor.name if nc.partition_id_tensor else None
    )
    in_names, out_names, out_avals, zero_outs = [], [], [], []
    for alloc in nc.m.functions[0].allocations:
        if not isinstance(alloc, mybir.MemoryLocationSet):
            continue
        name = alloc.memorylocations[0].name
        if alloc.kind == "ExternalInput":
            if name != partition_name:
                in_names.append(name)
        elif alloc.kind == "ExternalOutput":
            shape = tuple(alloc.tensor_shape)
            dtype = mybir.dt.np(alloc.dtype)
            out_avals.append(jax.core.ShapedArray(shape, dtype))
            out_names.append(name)
            zero_outs.append(np.zeros(shape, dtype))
    n_params = len(in_names)
    n_outs = len(out_avals)
    all_in_names = in_names + out_names
    if partition_name is not None:
        all_in_names = all_in_names + [partition_name]

    def _body(*args):
        operands = list(args)
        if partition_name is not None:
            operands.append(partition_id_tensor())
        outs = _bass_exec_p.bind(
            *operands,
            out_avals=tuple(out_avals),
            in_names=tuple(all_in_names),
            out_names=tuple(out_names),
            lowering_input_output_aliases=(),
            sim_require_finite=True,
            sim_require_nnan=True,
            nc=nc,
        )
        return tuple(outs)

    devices = jax.devices()[:NCORES]
    assert len(devices) == NCORES, f"need {NCORES} cores, got {len(devices)}"
    mesh = Mesh(np.asarray(devices), ("core",))
    in_specs = (PartitionSpec("core"),) * (n_params + n_outs)
    out_specs = (PartitionSpec("core"),) * n_outs
    sharded = jax.jit(
        shard_map(
            _body, mesh=mesh, in_specs=in_specs, out_specs=out_specs,
            check_rep=False,
        ),
        keep_unused=True,
    )

    def make_args(in_maps, on_device=False):
        concat_in = [
            np.concatenate([np.asarray(m[name]) for m in in_maps], axis=0)
            for name in in_names
        ]
        concat_zeros = [
            np.zeros((NCORES * z.shape[0], *z.shape[1:]), z.dtype)
            for z in zero_outs
        ]
        args = concat_in + concat_zeros
        if on_device:
            from jax.sharding import NamedSharding

            sh = NamedSharding(mesh, PartitionSpec("core"))
            args = [jax.device_put(a, sh) for a in args]
            jax.block_until_ready(args)
        return args

    def unpack(out_arrs):
        return [
            {
                name: np.asarray(out_arrs[i]).reshape(
                    NCORES, *out_avals[i].shape
                )[c]
                for i, name in enumerate(out_names)
            }
            for c in range(NCORES)
        ]

    def run(in_maps, blocking=True):
        out_arrs = sharded(*make_args(in_maps))
        if not blocking:
            return out_arrs
        return unpack(out_arrs)

    run.sharded = sharded
    run.make_args = make_args
    run.unpack = unpack
    _CACHE["runner"] = run
    return run


# ---------------------------------------------------------------- host logic


def _prepare_in_maps(ranking, gt):
    x = np.asarray(ranking, dtype=np.float32)
    gtv = np.asarray(gt).astype(np.int64)
    assert x.shape == (N, D), x.shape

    norms = np.linalg.norm(x, axis=1, keepdims=True).astype(np.float32)
    xn = (x / np.clip(norms, EPS, None)).astype(np.float32)
    xnt = np.ascontiguousarray(xn.T)

    g = np.abs(gtv[None, :] - gtv[:, None]).astype(np.float32)
    rel = (np.exp2(np.clip(10.0 - g, 0.0, None)) - 1.0).astype(np.float32)
    rel[np.arange(N), np.arange(N)] = 0.0

    i48 = np.eye(RPC, dtype=np.float32)
    ones128 = np.ones((128, 1), dtype=np.float32)

    in_maps = []
    for c in range(NCORES):
        n0 = c * RPC
        xst = np.ascontiguousarray(xn[n0 : n0 + RPC].T)
        dmask = np.zeros((RPC, N), dtype=np.float32)
        dmask[np.arange(RPC), n0 + np.arange(RPC)] = NEG_BIG
        relt = np.ascontiguousarray(
            rel[n0 : n0 + RPC].T * np.float32(LN2)
        )
        in_maps.append(
            {
                "xnt": xnt,
                "xst": xst,
                "i48": i48,
                "ones128": ones128,
                "dmask": dmask,
                "relt": relt,
            }
        )
    return in_maps, gtv


def _idcg_per_row(gtv):
    """idcg depends only on gt[n]; reproduce the reference's sorted-rel sum."""
    M = N - 1
    disc = np.log2(np.arange(M, dtype=np.float32) + 2.0).astype(np.float32)
    maxv = int(gtv.max())
    hist = np.bincount(gtv, minlength=maxv + 1)
    idcg_by_val = {}
    for a in np.unique(gtv):
        a = int(a)
        chunks = []
        d = 0
        while True:
            if d == 0:
                cnt = hist[a] - 1
            else:
                cnt = 0
                if a - d >= 0:
                    cnt += hist[a - d]
                if a + d <= maxv:
                    cnt += hist[a + d]
                if a - d < 0 and a + d > maxv:
                    break
            v = np.float32(2.0 ** max(10.0 - d, 0.0) - 1.0)
            chunks.append(np.full(cnt, v, dtype=np.float32))
            d += 1
        rel_sorted = np.concatenate(chunks)
        assert rel_sorted.shape == (M,)
        idcg_by_val[a] = np.float32(
            np.sum((rel_sorted / disc).astype(np.float32), dtype=np.float32)
        )
    return np.array([idcg_by_val[int(a)] for a in gtv], dtype=np.float32)


def _finalize(dcg, gtv):
    idcg = _idcg_per_row(gtv)
    valid = idcg != 0.0
    ndcg = np.where(
        valid, dcg / np.where(valid, idcg, np.float32(1.0)), np.float32(0.0)
    ).astype(np.float32)
    cnt = int(valid.sum())
    if cnt == 0:
        return np.float32(1.0)
    mean = np.float32(ndcg.sum(dtype=np.float32) / np.float32(max(cnt, 1)))
    return np.float32(np.float32(1.0) - mean)


def kernel(ranking, gt):
    in_maps, gtv = _prepare_in_maps(ranking, gt)
    run = _get_runner()
    results = run(in_maps)
    dcg = np.concatenate(
        [np.asarray(results[c]["dcg"]).reshape(-1) for c in range(NCORES)]
    ).astype(np.float32)
    return _finalize(dcg, gtv)


# revision 16
# speedup vs baseline: 33.3483x; 33.3483x over previous
"""Trainium2 Bass kernel for nn_DGCLoss (DCG/NDCG ranking loss).

Math restructure (vs. the reference's [N, M, M-1] cube):

For row n, with s = off-diag cosine-sim row mapped to [0,1]:
    indicator[n,i] = 1 + sum_{j != i} sigmoid((s_j - s_i)/K)
Working instead with the FULL 384-vector of cosines c (diag masked to -1e6):
    T(i) = sum_{j=0..N-1} sigmoid(500*(c_j - c_i))      # diag j=n contributes 0
         = indicator[n,i] - 0.5                         # (j=i term = 0.5)
Using sigmoid(z) = 0.5 + 0.5*tanh(z/2):
    A(i) = sum_j tanh(250*(c_j - c_i));  u = indicator+1 = 0.5*A + N/2 + 1.5
    dcg[n] = sum_i rel0[n,i] * ln2 / ln(u(i))           # rel0 has zero diag

Per core (8 cores, 48 rows each):
  - PE: gram slice (fp32r), s~ transposes, one broadcast matmul per row
        (one-hot weights x s~ tile -> PSUM[p,j] = 250*c_j), final reduction.
  - ACT: 3 tanh instructions per row ([128,384], per-partition bias
        -250*c_i, accum_out = row sums) -> A tiles; tail ln.
  - DVE: diag mask add, bias scaling, reciprocal, relevance multiply.
idcg depends only on gt[n] (<= 6 distinct values): computed on host.
"""

import math

import numpy as np

N = 384
D = 256
NCORES = 8
RPC = N // NCORES  # 48 rows per core
EPS = 1e-8
NEG_BIG = -1.0e6
LN2 = math.log(2.0)

_CACHE = {}


# ---------------------------------------------------------------- device code


def _build_nc():
    """Build + compile the (SPMD, per-core) Bass program."""
    from contextlib import ExitStack

    import concourse.bacc as bacc
    import concourse.mybir as mybir
    import concourse.tile as tile

    f32 = mybir.dt.float32
    AF = mybir.ActivationFunctionType

    nc = bacc.Bacc(
        "TRN2",
        target_bir_lowering=False,
        debug=False,
        enable_asserts=True,
        num_devices=NCORES,
    )

    xnt_d = nc.dram_tensor("xnt", [D, N], f32, kind="ExternalInput")
    xst_d = nc.dram_tensor("xst", [D, RPC], f32, kind="ExternalInput")
    i48_d = nc.dram_tensor("i48", [RPC, RPC], f32, kind="ExternalInput")
    ones_d = nc.dram_tensor("ones128", [128, 1], f32, kind="ExternalInput")
    dmask_d = nc.dram_tensor("dmask", [RPC, N], f32, kind="ExternalInput")
    relt_d = nc.dram_tensor("relt", [N, RPC], f32, kind="ExternalInput")
    dcg_d = nc.dram_tensor("dcg", [1, RPC], f32, kind="ExternalOutput")

    with tile.TileContext(nc) as tc, ExitStack() as ctx:
        const = ctx.enter_context(tc.tile_pool(name="const", bufs=1))
        junkp = ctx.enter_context(tc.tile_pool(name="junk", bufs=2))

        xnt_sb = []
        for k in range(2):
            t = const.tile([128, N], f32, name=f"xnt_sb{k}", tag=f"xnt{k}")
            nc.sync.dma_start(t[:], xnt_d.ap()[128 * k : 128 * (k + 1), :])
            xnt_sb.append(t)
        xst_sb = []
        for k in range(2):
            t = const.tile([128, RPC], f32, name=f"xst_sb{k}", tag=f"xst{k}")
            nc.sync.dma_start(t[:], xst_d.ap()[128 * k : 128 * (k + 1), :])
            xst_sb.append(t)
        i48_sb = const.tile([RPC, RPC], f32, name="i48_sb", tag="i48")
        nc.sync.dma_start(i48_sb[:], i48_d.ap()[:])
        # one-hot broadcast weights built on device: W[r, 128r:128(r+1)] = 250
        # via W_block_r = 250 * i48[:, r]  (per-partition scalar multiply)
        w250_sb = const.tile([RPC, RPC * 128], f32, name="w250_sb", tag="w250")
        c250_sb = const.tile([RPC, 128], f32, name="c250_sb", tag="c250")
        nc.vector.memset(c250_sb[:], 250.0)
        for r in range(RPC):
            nc.vector.tensor_scalar_mul(
                w250_sb[:, 128 * r : 128 * (r + 1)],
                c250_sb[:],
                i48_sb[:, r : r + 1],
            )
        ones_sb = const.tile([128, 1], f32, name="ones_sb", tag="ones")
        nc.sync.dma_start(ones_sb[:], ones_d.ap()[:])
        dmask_sb = const.tile([RPC, N], f32, name="dmask_sb", tag="dmask")
        nc.sync.dma_start(dmask_sb[:], dmask_d.ap()[:])
        relt_sb = []
        for c in range(3):
            t = const.tile([128, RPC], f32, name=f"relt_sb{c}", tag=f"relt{c}")
            nc.sync.dma_start(t[:], relt_d.ap()[128 * c : 128 * (c + 1), :])
            relt_sb.append(t)

        s_sb = const.tile([RPC, N], f32, name="s_sb", tag="s")
        bt_sb = [
            const.tile([128, RPC], f32, name=f"bt_sb{c}", tag=f"bt{c}")
            for c in range(3)
        ]
        a_sb = [
            const.tile([128, RPC], f32, name=f"a_sb{c}", tag=f"a{c}")
            for c in range(3)
        ]

        # ---- phase 1: gram slice, diag mask, transposed bias tiles
        with tc.tile_pool(name="pg", bufs=1, space="PSUM") as pgp, tc.tile_pool(
            name="pt", bufs=1, space="PSUM"
        ) as ptp:
            pg = pgp.tile([RPC, N], f32, name="pg", tag="pg")
            nc.tensor.matmul(
                pg[:],
                xst_sb[0][:],
                xnt_sb[0][:],
                start=True,
                stop=False,
            )
            nc.tensor.matmul(
                pg[:],
                xst_sb[1][:],
                xnt_sb[1][:],
                start=False,
                stop=True,
            )
            # s~ = cosine row block + diagonal mask (-1e6 at global diag)
            nc.vector.tensor_add(s_sb[:], pg[:], dmask_sb[:])
            for c in range(3):
                pt = ptp.tile([128, RPC], f32, name=f"pt{c}", tag=f"pt{c}")
                nc.tensor.transpose(
                    pt[:], s_sb[0:RPC, 128 * c : 128 * (c + 1)], i48_sb[:]
                )
                # bias = -250 * s~^T  (per-partition bias for the tanh)
                nc.vector.tensor_scalar_mul(bt_sb[c][:], pt[:], -250.0)

        # ---- phase 2: main loop. Per row r: one broadcast matmul + 3 tanh
        last_tanh = None
        with tc.tile_pool(name="pb", bufs=6, space="PSUM") as pbp:
            for r in range(RPC):
                pb = pbp.tile([128, N], f32, name=f"pb{r}", tag="pb")
                # pb[p, j] = 250 * s~[r, j]  (one-hot row r of w250)
                nc.tensor.matmul(
                    pb[:],
                    w250_sb[:, 128 * r : 128 * (r + 1)],
                    s_sb[:],
                    start=True,
                    stop=True,
                )
                for c in range(3):
                    jt = junkp.tile([128, N], f32, name=f"jt{r}_{c}", tag="junk")
                    last_tanh = nc.scalar.activation(
                        jt[:],
                        pb[:],
                        AF.Tanh,
                        bias=bt_sb[c][:, r : r + 1],
                        scale=1.0,
                        accum_out=a_sb[c][:, r : r + 1],
                    )

        # ---- phase 3: tail.  u = 0.5*A + (N/2 + 1.5); dcg = sum_i relt/ln(u)
        with tc.tile_pool(name="pd", bufs=1, space="PSUM") as pdp:
            pd = pdp.tile([1, RPC], f32, name="pd", tag="pd")
            lnu = [
                const.tile([128, RPC], f32, name=f"lnu{c}", tag=f"lnu{c}")
                for c in range(3)
            ]
            dterm = [
                const.tile([128, RPC], f32, name=f"dterm{c}", tag=f"dterm{c}")
                for c in range(3)
            ]
            ubias = const.tile([128, 1], f32, name="ubias", tag="ubias")
            nc.vector.memset(ubias[:], float(N / 2 + 1.5))
            for c in range(3):
                ln_inst = nc.scalar.activation(
                    lnu[c][:],
                    a_sb[c][:],
                    AF.Ln,
                    bias=ubias[:],
                    scale=0.5,
                )
                # keep the Ln (different ACT table set) strictly after every
                # tanh so only one table swap happens
                tile.add_dep_helper(
                    ln_inst.ins,
                    last_tanh.ins,
                    reason="batch ACT table sets: all tanh before ln",
                )
                nc.vector.reciprocal(lnu[c][:], lnu[c][:])
                nc.vector.tensor_mul(dterm[c][:], lnu[c][:], relt_sb[c][:])
                nc.tensor.matmul(
                    pd[:],
                    ones_sb[:],
                    dterm[c][:],
                    start=(c == 0),
                    stop=(c == 2),
                )
            out_sb = const.tile([1, RPC], f32, name="out_sb", tag="out")
            nc.vector.tensor_copy(out_sb[:], pd[:])
            nc.sync.dma_start(dcg_d.ap()[:], out_sb[:])

    nc.compile()
    return nc


def _get_nc():
    if "nc" not in _CACHE:
        _CACHE["nc"] = _build_nc()
    return _CACHE["nc"]


# ------------------------------------------------------------------ execution


def _get_runner():
    """Cached jitted 8-core SPMD executor (modeled on bass2jax's
    run_bass_via_pjrt multi-core path, but reusable across calls)."""
    if "runner" in _CACHE:
        return _CACHE["runner"]

    import jax
    from jax.sharding import Mesh, PartitionSpec
    from jax.experimental.shard_map import shard_map

    import concourse.mybir as mybir
    from concourse.bass2jax import (
        _bass_exec_p,
        install_neuronx_cc_hook,
        partition_id_tensor,
    )

    nc = _get_nc()
    install_neuronx_cc_hook()

    partition_name = (
        nc.partition_id_tensor.name if nc.partition_id_tensor else None
    )
    in_names, out_names, out_avals, zero_outs = [], [], [], []
    for alloc in nc.m.functions[0].allocations:
        if not isinstance(alloc, mybir.MemoryLocationSet):
            continue
        name = alloc.memorylocations[0].name
        if alloc.kind == "ExternalInput":
            if name != partition_name:
                in_names.append(name)
        elif alloc.kind == "ExternalOutput":
            shape = tuple(alloc.tensor_shape)
            dtype = mybir.dt.np(alloc.dtype)
            out_avals.append(jax.core.ShapedArray(shape, dtype))
            out_names.append(name)
            zero_outs.append(np.zeros(shape, dtype))
    n_params = len(in_names)
    n_outs = len(out_avals)
    all_in_names = in_names + out_names
    if partition_name is not None:
        all_in_names = all_in_names + [partition_name]

    def _body(*args):
        operands = list(args)
        if partition_name is not None:
            operands.append(partition_id_tensor())
        outs = _bass_exec_p.bind(
            *operands,
            out_avals=tuple(out_avals),
            in_names=tuple(all_in_names),
            out_names=tuple(out_names),
            lowering_input_output_aliases=(),
            sim_require_finite=True,
            sim_require_nnan=True,
            nc=nc,
        )
        return tuple(outs)

    devices = jax.devices()[:NCORES]
    assert len(devices) == NCORES, f"need {NCORES} cores, got {len(devices)}"
    mesh = Mesh(np.asarray(devices), ("core",))
    in_specs = (PartitionSpec("core"),) * (n_params + n_outs)
    out_specs = (PartitionSpec("core"),) * n_outs
    sharded = jax.jit(
        shard_map(
            _body, mesh=mesh, in_specs=in_specs, out_specs=out_specs,
            check_rep=False,
        ),
        keep_unused=True,
    )

    def make_args(in_maps, on_device=False):
        concat_in = [
            np.concatenate([np.asarray(m[name]) for m in in_maps], axis=0)
            for name in in_names
        ]
        concat_zeros = [
            np.zeros((NCORES * z.shape[0], *z.shape[1:]), z.dtype)
            for z in zero_outs
        ]
        args = concat_in + concat_zeros
        if on_device:
            from jax.sharding import NamedSharding

            sh = NamedSharding(mesh, PartitionSpec("core"))
            args = [jax.device_put(a, sh) for a in args]
            jax.block_until_ready(args)
        return args

    def unpack(out_arrs):
        return [
            {
                name: np.asarray(out_arrs[i]).reshape(
                    NCORES, *out_avals[i].shape
                )[c]
                for i, name in enumerate(out_names)
            }
            for c in range(NCORES)
        ]

    def run(in_maps, blocking=True):
        out_arrs = sharded(*make_args(in_maps))
        if not blocking:
            return out_arrs
        return unpack(out_arrs)

    run.sharded = sharded
    run.make_args = make_args
    run.unpack = unpack
    _CACHE["runner"] = run
    return run


# ---------------------------------------------------------------- host logic


def _prepare_in_maps(ranking, gt):
    x = np.asarray(ranking, dtype=np.float32)
    gtv = np.asarray(gt).astype(np.int64)
    assert x.shape == (N, D), x.shape

    norms = np.linalg.norm(x, axis=1, keepdims=True).astype(np.float32)
    xn = (x / np.clip(norms, EPS, None)).astype(np.float32)
    xnt = np.ascontiguousarray(xn.T)

    g = np.abs(gtv[None, :] - gtv[:, None]).astype(np.float32)
    rel = (np.exp2(np.clip(10.0 - g, 0.0, None)) - 1.0).astype(np.float32)
    rel[np.arange(N), np.arange(N)] = 0.0

    i48 = np.eye(RPC, dtype=np.float32)
    ones128 = np.ones((128, 1), dtype=np.float32)

    in_maps = []
    for c in range(NCORES):
        n0 = c * RPC
        xst = np.ascontiguousarray(xn[n0 : n0 + RPC].T)
        dmask = np.zeros((RPC, N), dtype=np.float32)
        dmask[np.arange(RPC), n0 + np.arange(RPC)] = NEG_BIG
        relt = np.ascontiguousarray(
            rel[n0 : n0 + RPC].T * np.float32(LN2)
        )
        in_maps.append(
            {
                "xnt": xnt,
                "xst": xst,
                "i48": i48,
                "ones128": ones128,
                "dmask": dmask,
                "relt": relt,
            }
        )
    return in_maps, gtv


def _idcg_per_row(gtv):
    """idcg depends only on gt[n]; reproduce the reference's sorted-rel sum."""
    M = N - 1
    disc = np.log2(np.arange(M, dtype=np.float32) + 2.0).astype(np.float32)
    gtv = gtv - gtv.min()  # |gt_i - gt_j| is shift-invariant; bincount needs >= 0
    maxv = int(gtv.max())
    hist = np.bincount(gtv, minlength=maxv + 1)
    idcg_by_val = {}
    for a in np.unique(gtv):
        a = int(a)
        chunks = []
        d = 0
        while True:
            if d == 0:
                cnt = hist[a] - 1
            else:
                cnt = 0
                if a - d >= 0:
                    cnt += hist[a - d]
                if a + d <= maxv:
                    cnt += hist[a + d]
                if a - d < 0 and a + d > maxv:
                    break
            v = np.float32(2.0 ** max(10.0 - d, 0.0) - 1.0)
            chunks.append(np.full(cnt, v, dtype=np.float32))
            d += 1
        rel_sorted = np.concatenate(chunks)
        assert rel_sorted.shape == (M,)
        idcg_by_val[a] = np.float32(
            np.sum((rel_sorted / disc).astype(np.float32), dtype=np.float32)
        )
    return np.array([idcg_by_val[int(a)] for a in gtv], dtype=np.float32)


def _finalize(dcg, gtv):
    idcg = _idcg_per_row(gtv)
    valid = idcg != 0.0
    ndcg = np.where(
        valid, dcg / np.where(valid, idcg, np.float32(1.0)), np.float32(0.0)
    ).astype(np.float32)
    cnt = int(valid.sum())
    if cnt == 0:
        return np.float32(1.0)
    mean = np.float32(ndcg.sum(dtype=np.float32) / np.float32(max(cnt, 1)))
    return np.float32(np.float32(1.0) - mean)


def kernel(ranking, gt):
    in_maps, gtv = _prepare_in_maps(ranking, gt)
    run = _get_runner()
    results = run(in_maps)
    dcg = np.concatenate(
        [np.asarray(results[c]["dcg"]).reshape(-1) for c in range(NCORES)]
    ).astype(np.float32)
    return _finalize(dcg, gtv)
